# revision 51
# baseline (speedup 1.0000x reference)
"""Multi-head attention (B=2, S=2048, D=1024, H=16) as an 8-core TRN2 Bass kernel.

Sharding: (batch, head-block) across cores. Core c handles batch c//4 and
heads (c%4)*4 .. (c%4)*4+3. Projections are column-sharded over heads;
the output projection is row-sharded (per-core partial sums, reduced on host).

v2 changes over the session-1 baseline:
  - diagonal score tiles trimmed to the causal columns (min 256 wide to
    dodge the fp32r small-moving penalty); exp waves trimmed to match
  - output projection emitted per 512-col query chunk as PE filler inside
    later attention units (the late units are ACT-exp-bound; out-proj
    matmuls keep the PE busy there) instead of all at the end
  - reciprocal Newton step dropped (DVE reciprocal ~1e-3 rel err, fine
    at the 2e-2 gate); denominator broadcast stays a DRAM round-trip DMA
    except for the very last unit, which uses a PE ones-outer-product
    broadcast so the tail never waits on DMA latency
  - last chunk's output projection split by contraction half: the first
    half runs as filler during the last attention unit, the second half
    (all that remains after the final norm) goes to a separate outT2
    partial that the host adds — shortens the serial tail
  - outT stored bf16 (halves output DMA), host sums partials in f32
  - note: GPSIMD (Pool) tensor ops / DMA issue measured ~4x slower on
    real HW than CoreSim models them — everything stays off GPSIMD
"""

import numpy as np

import concourse.bass as bass
import concourse.bacc as bacc
import concourse.mybir as mybir
import concourse.tile as tile
from concourse.bass_utils import run_bass_kernel_spmd

B, S, D, H = 2, 2048, 1024, 16
HD = D // H            # 64
NCORES = 8
CPB = NCORES // B      # cores per batch: 4
HPC = H // CPB         # heads per core: 4
DH = HPC * HD          # 256 per-core head dims
P = 128
QCW = 512              # query chunk width
NQC = S // QCW         # 4
NKT = S // P           # 16 key tiles
KC = D // P            # 8 contraction chunks
F32 = mybir.dt.float32
F32R = mybir.dt.float32r
BF16 = mybir.dt.bfloat16
Exp = mybir.ActivationFunctionType.Exp
Identity = mybir.ActivationFunctionType.Identity
Copy = mybir.ActivationFunctionType.Copy
MULT = mybir.AluOpType.mult
ADD = mybir.AluOpType.add

_CACHE = {}

# winning build config (kept in sync with kernel() gather logic)
BEST = dict(LAG=4, SPLIT_LAST=False, EVAC_LAST='alt', PAIR_LAST=True,
            FINETRIM=True)


def _r(ap):
    if ap.dtype == BF16:
        return ap
    return ap.bitcast(F32R)


def build(dbg=False, reps=1, PWB=4, BCB=3, OSB=8, WKT=2, mask_eng='vector',
          SCB=2, LAG=3, DIST=(5, 1, 1, 1), HORD=(0, 1, 3, 2), TRIM=True,
          NEWTON=False, OUT_DT=BF16, SB_BCAST=False, FPW=1, OPB=4,
          EVAC_ENG='vector', DMAQ='sync', BCPE='last', SPLIT_LAST=True,
          QK_EVAC='vector', BC_ENG='vector', XSPLIT=True, IN_DT=BF16,
          TAILFILL=False, RESV=0, PAIR_STORE=False, PAIR_LAST=False,
          WARM_HEAD=0, WARM_TAIL=0, EVAC_LAST=None, FINETRIM=False,
          OUTQ=((), (), (0,), (1, 2)), dummy=None, dummy_n=3):
    nc = bacc.Bacc("TRN2", target_bir_lowering=False, debug=False,
                   num_devices=NCORES)

    xin = "xT16" if IN_DT == BF16 else "xT"
    win = "16" if IN_DT == BF16 else ""
    xT_d = nc.dram_tensor(xin, [D, S], IN_DT, kind="ExternalInput").ap()
    wqT_d = nc.dram_tensor("wqT" + win, [D, DH], IN_DT, kind="ExternalInput").ap()
    wkT_d = nc.dram_tensor("wkT" + win, [D, DH], IN_DT, kind="ExternalInput").ap()
    wvT_d = nc.dram_tensor("wvT" + win, [D, DH], IN_DT, kind="ExternalInput").ap()
    woT_d = nc.dram_tensor("woT", [DH, D], F32R, kind="ExternalInput").ap()
    bq_d = nc.dram_tensor("bq2", [P, 2], F32, kind="ExternalInput").ap()
    bk_d = nc.dram_tensor("bk2", [P, 2], F32, kind="ExternalInput").ap()
    bv_d = nc.dram_tensor("bv1", [1, DH], F32, kind="ExternalInput").ap()
    tri_d = nc.dram_tensor("tri", [P, P], F32, kind="ExternalInput").ap()
    one_d = nc.dram_tensor("one64", [1, NKT * HPC], F32R, kind="ExternalInput").ap()
    scr_d = nc.dram_tensor("rscratch", [HPC * NQC, QCW], F32R, kind="Internal").ap()
    outT_d = nc.dram_tensor("outT", [D, S], OUT_DT, kind="ExternalOutput").ap()
    outT2_d = (nc.dram_tensor("outT2", [D, QCW], OUT_DT, kind="ExternalOutput").ap()
               if SPLIT_LAST else None)
    if dbg:
        dbg_q = nc.dram_tensor("dbg_q", [2 * P, S], F32, kind="ExternalOutput").ap()
        dbg_k = nc.dram_tensor("dbg_k", [2 * P, S], F32, kind="ExternalOutput").ap()
        dbg_v = nc.dram_tensor("dbg_v", [P, NKT * (DH + HPC)], F32, kind="ExternalOutput").ap()
        dbg_o = nc.dram_tensor("dbg_o", [2 * P, S], F32, kind="ExternalOutput").ap()

    with tile.TileContext(nc) as tc:
        # ---- persistent SBUF tensors ----
        _frees = []
        xT_sb, _f = tc.tile([P, KC * S], IN_DT, name="xT_sb"); _frees.append(_f)         # 32KB/part bf16
        wq_sb, _f = tc.tile([P, KC * DH], IN_DT, name="wq_sb"); _frees.append(_f)        # 4KB
        wk_sb, _f = tc.tile([P, KC * DH], IN_DT, name="wk_sb"); _frees.append(_f)
        wv_sb, _f = tc.tile([P, KC * DH], IN_DT, name="wv_sb"); _frees.append(_f)
        wo_sb, _f = tc.tile([P, 2 * D], F32R, name="wo_sb"); _frees.append(_f)           # 8KB
        qT_sb, _f = tc.tile([P, 2 * S], F32R, name="qT_sb"); _frees.append(_f)           # 16KB (m-chunks)
        kT_sb, _f = tc.tile([P, 2 * S], F32R, name="kT_sb"); _frees.append(_f)
        v_sb, _f = tc.tile([P, NKT * (DH + HPC)], F32R, name="v_sb"); _frees.append(_f)  # [128, 16*260]
        oTn_sb, _f = tc.tile([P, 2 * S], F32R, name="oTn_sb"); _frees.append(_f)          # normalized attn outT
        tri_sb, _f = tc.tile([P, P], F32, name="tri_sb"); _frees.append(_f)
        bq_sb, _f = tc.tile([P, 2], F32, name="bq_sb"); _frees.append(_f)
        bk_sb, _f = tc.tile([P, 2], F32, name="bk_sb"); _frees.append(_f)
        bv_sb, _f = tc.tile([P, DH], F32, name="bv_sb"); _frees.append(_f)              # broadcast bv

        # ---- input DMAs, ordered by first consumer. Small tensors (biases,
        # tri, ones) first: the q/k bias-add evac needs them and they cost
        # ~nothing; queueing them after the 8MB of x blocks stalled the whole
        # pipeline ~20us. Then wq/wk/x0 interleaved per-kc so the first
        # projection matmuls start after ~1.5MB instead of 4MB.
        ones_sb, _f = tc.tile([1, HD], F32R, name="ones_sb"); _frees.append(_f)
        w3 = [(w_sb[:].rearrange("p (kc d) -> p kc d", kc=KC),
               w_d.rearrange("(kc p) d -> p kc d", p=P))
              for w_sb, w_d in ((wq_sb, wqT_d), (wk_sb, wkT_d), (wv_sb, wvT_d))]
        def load_w(i):
            nc.sync.dma_start(out=w3[i][0][:], in_=w3[i][1][:])
        xT3o = xT_sb[:].rearrange("p (kc s) -> p kc s", kc=KC)
        xT3i = xT_d.rearrange("(kc p) s -> p kc s", p=P)
        def load_x(n, kc=None):
            if kc is None:
                nc.sync.dma_start(out=xT3o[:, :, n * QCW:(n + 1) * QCW],
                                  in_=xT3i[:, :, n * QCW:(n + 1) * QCW])
            else:
                nc.sync.dma_start(
                    out=xT3o[:, kc, n * QCW:(n + 1) * QCW],
                    in_=xT3i[:, kc, n * QCW:(n + 1) * QCW])
        # head order: exactly what the first q/k projection matmuls and their
        # bias-evac need, then the rest; wv trails (v-groups run as pre-PV
        # fill inside the first unit), tri/bv/ones before their ~15us uses.
        # Few, large DMAs: each dma_start costs ~625ns of serialized HWDGE
        # descriptor-gen, so per-kc splitting is counterproductive.
        def load_x_half(n, h):
            nc.sync.dma_start(
                out=xT3o[:, h * (KC // 2):(h + 1) * (KC // 2),
                         n * QCW:(n + 1) * QCW],
                in_=xT3i[:, h * (KC // 2):(h + 1) * (KC // 2),
                         n * QCW:(n + 1) * QCW])
        def load_w_half(i, h):
            nc.sync.dma_start(out=w3[i][0][:, h * (KC // 2):(h + 1) * (KC // 2)],
                              in_=w3[i][1][:, h * (KC // 2):(h + 1) * (KC // 2)])
        def load_x_q(n, q):
            nc.sync.dma_start(
                out=xT3o[:, 2 * q:2 * q + 2, n * QCW:(n + 1) * QCW],
                in_=xT3i[:, 2 * q:2 * q + 2, n * QCW:(n + 1) * QCW])
        load_w_half(0, 0)
        load_x_q(0, 0)
        load_w_half(1, 0)
        load_x_q(0, 1)
        load_w_half(0, 1)
        load_x_q(0, 2)
        load_w_half(1, 1)
        load_x_q(0, 3)
        nc.sync.dma_start(out=bq_sb[:], in_=bq_d[:])
        nc.sync.dma_start(out=bk_sb[:], in_=bk_d[:])
        nc.sync.dma_start(out=tri_sb[:], in_=tri_d[:])
        nc.sync.dma_start(out=bv_sb[:], in_=bv_d[0:1, :].to_broadcast((P, DH)))
        nc.sync.dma_start(out=ones_sb[:], in_=one_d[0:1, 0:HD])
        load_w(2)
        wo3o = wo_sb[:].rearrange("p (ac d) -> p ac d", ac=2)
        wo3i = woT_d.rearrange("(ac p) d -> p ac d", p=P)
        def load_wo(ac):
            nc.sync.dma_start(out=wo3o[:, ac], in_=wo3i[:, ac])
        if not XSPLIT:
            for n in range(1, NQC):
                load_x(n)
            load_wo(0)
            load_wo(1)
        dmaq = getattr(nc, DMAQ)

        # bottleneck-probe scratch: dummy ops interleaved between attention
        # units load one engine without data deps on real work
        if dummy:
            dumm_sb, _f = tc.tile([P, QCW], F32, name="dumm_sb"); _frees.append(_f)

        from contextlib import nullcontext
        with (
            tc.tile_pool(name="ps_score", bufs=SCB, space="PSUM") as ps_score,
            tc.tile_pool(name="ps_o", bufs=OPB, space="PSUM") as ps_o,
            (tc.tile_pool(name="ps_dum", bufs=1, space="PSUM")
             if dummy == 'pe' else nullcontext()) as ps_dum,
            tc.tile_pool(name="pw", bufs=PWB) as pw_pool,
            tc.tile_pool(name="bcast", bufs=BCB) as bcast_pool,
            tc.tile_pool(name="recip", bufs=BCB) as recip_pool,
            tc.tile_pool(name="tmp", bufs=2) as tmp_pool,
            tc.tile_pool(name="outst", bufs=OSB) as outst_pool,
        ):
            def emit_dummies():
                for _ in range(dummy_n):
                    if dummy == 'act':
                        nc.scalar.activation(dumm_sb[:], xT_sb[:, 0:QCW], Exp)
                    elif dummy == 'dve':
                        nc.vector.tensor_copy(dumm_sb[:], xT_sb[:, 0:QCW])
                    elif dummy == 'pe':
                        dps = ps_dum.tile([P, QCW], F32, tag="dum", name="dps")
                        nc.tensor.matmul(dps[:], _r(wq_sb[:, 0:P]),
                                         _r(xT_sb[:, 0:QCW]),
                                         start=True, stop=True)
            v3 = v_sb.rearrange("p (t c) -> p t c", c=HD + 1)  # [128,64,65]
            # ones column per head-block for the softmax denominator
            nc.sync.dma_start(out=v3[:, :, HD],
                              in_=one_d[0:1, :].to_broadcast((P, NKT * HPC)))
            for _rep in range(reps):

                def proj_qk_group(dst, w_sb, b_sb, m, n):
                    ps = ps_o.tile([P, QCW], F32, tag="oT", name="ps_a")
                    for kc in range(KC):
                        nc.tensor.matmul(
                            ps[:],
                            _r(w_sb[:, kc * DH + m * P: kc * DH + (m + 1) * P]),
                            _r(xT_sb[:, kc * S + n * QCW: kc * S + (n + 1) * QCW]),
                            start=(kc == 0), stop=(kc == KC - 1))
                    if QK_EVAC == 'scalar':
                        nc.scalar.activation(
                            dst[:, m * S + n * QCW: m * S + (n + 1) * QCW],
                            ps[:], Identity, bias=b_sb[:, m:m + 1])
                    else:
                        nc.vector.tensor_scalar(
                            out=dst[:, m * S + n * QCW: m * S + (n + 1) * QCW],
                            in0=ps[:], scalar1=b_sb[:, m:m + 1],
                            scalar2=None, op0=ADD)

                def proj_v_group(sb):
                    ps = ps_o.tile([P, QCW], F32, tag="oT", name="ps_v")[:, :DH]
                    for kc in range(KC):
                        nc.tensor.matmul(
                            ps[:],
                            _r(xT_sb[:, kc * S + sb * P: kc * S + sb * P + P]),
                            _r(wv_sb[:, kc * DH:(kc + 1) * DH]),
                            start=(kc == 0), stop=(kc == KC - 1))
                    nc.vector.tensor_tensor(
                        out=v3[:, sb * HPC:(sb + 1) * HPC, 0:HD],
                        in0=ps[:].rearrange("p (l d) -> p l d", d=HD),
                        in1=bv_sb[:].rearrange("p (l d) -> p l d", d=HD),
                        op=ADD)

                def proj_groups(n):
                    gs = []
                    for dst, w_sb, b_sb in ((qT_sb, wq_sb, bq_sb),
                                            (kT_sb, wk_sb, bk_sb)):
                        for m in range(2):
                            gs.append(lambda d=dst, w=w_sb, b=b_sb, mm=m:
                                      proj_qk_group(d, w, b, mm, n))
                    for sb in range(HPC * n, HPC * (n + 1)):
                        gs.append(lambda s=sb: proj_v_group(s))
                    return gs

                def emit_pv(st, pw, w0, w1):
                    for kt in range(w0, w1):
                        j = kt - w0
                        o = max(0, kt * P - st["q0"])
                        nc.tensor.matmul(
                            st["oT"][0:HD + 1, o:QCW],
                            _r(v3[:, kt * HPC + st["lh"], 0:HD + 1]),
                            _r(pw[:, j * QCW + o:(j + 1) * QCW]),
                            start=(kt == 0), stop=(kt == st["nk"] - 1))

                def emit_norm(st, pe_bcast=False):
                    lh, hp, e, q0, qc, oT = (st["lh"], st["hp"], st["e"],
                                             st["q0"], st["qc"], st["oT"])
                    recip = recip_pool.tile([2, QCW], F32R, tag="recip")
                    den = oT[HD:HD + 1, 0:QCW]
                    r1 = recip[0:1, 0:QCW]
                    with nc.allow_low_precision(reason="f32r recip feeds "
                                                  "f32r broadcast matmul"):
                        nc.vector.reciprocal(r1, den)
                    if NEWTON:
                        t = recip[1:2, 0:QCW]
                        nc.vector.tensor_tensor(out=t, in0=den, in1=r1, op=MULT)
                        nc.vector.tensor_scalar(out=t, in0=t, scalar1=-1.0,
                                                scalar2=2.0, op0=MULT, op1=ADD)
                        nc.vector.tensor_tensor(out=r1, in0=r1, in1=t, op=MULT)
                    if pe_bcast:
                        # broadcast across partitions on the PE (ones outer
                        # product) — skips the DMA round-trip latency (and,
                        # when used for all units, keeps the norm off the DMA
                        # queue entirely so oT PSUM frees promptly)
                        bc_ps = ps_score.tile([P, WKT * QCW], F32, tag="score",
                                              name="bc_ps")
                        nc.tensor.matmul(bc_ps[0:HD, 0:QCW], _r(ones_sb[:]),
                                         _r(r1), start=True, stop=True)
                        # DVE cannot read two PSUM operands: stage to SBUF
                        bc = bcast_pool.tile([HD, QCW], F32R, tag="bcast")
                        if BC_ENG == 'scalar':
                            nc.scalar.activation(bc[:], bc_ps[0:HD, 0:QCW],
                                                 Copy)
                        else:
                            nc.vector.tensor_copy(bc[:], bc_ps[0:HD, 0:QCW])
                        bc = bc[:]
                    else:
                        bc = bcast_pool.tile([HD, QCW], F32R, tag="bcast")
                        srow = lh * NQC + qc
                        nc.sync.dma_start(out=scr_d[srow:srow + 1, :], in_=r1)
                        nc.sync.dma_start(
                            out=bc[:],
                            in_=scr_d[srow:srow + 1, :].to_broadcast((HD, QCW)))
                    if e == 0:
                        nc.vector.tensor_tensor(
                            out=oTn_sb[0:HD, hp * S + q0: hp * S + q0 + QCW],
                            in0=oT[0:HD, 0:QCW], in1=bc[:], op=MULT)
                    else:
                        tmp = tmp_pool.tile([HD, QCW], F32R, tag="tmp")
                        nc.vector.tensor_tensor(
                            out=tmp[:], in0=oT[0:HD, 0:QCW], in1=bc[:], op=MULT)
                        dmaq.dma_start(
                            out=oTn_sb[HD:P, hp * S + q0: hp * S + q0 + QCW],
                            in_=tmp[:])

                def emit_attn_unit(qc, lh, fillq, pace, cred, pe_bcast=False,
                                  prepv=None):
                    # QK+exp waves with PV lagging LAG waves behind; fillq
                    # thunks are popped between waves to keep the PE busy
                    # while ACT chews the exp backlog. prepv thunks MUST all
                    # run before this unit's first emit_pv (tile deps are
                    # emission-ordered), so they're force-drained then.
                    hp, e = lh // 2, lh % 2
                    prow = slice(e * 64, (e + 1) * 64)
                    nk = HPC * (qc + 1)
                    q0 = qc * QCW
                    st = {"lh": lh, "hp": hp, "e": e, "qc": qc, "q0": q0,
                          "nk": nk, "oT": ps_o.tile([P, QCW], F32, tag="oT",
                                                    name="oT")}
                    pend = []
                    def drain_prepv():
                        while prepv:
                            prepv.pop(0)()
                    for w0 in range(0, nk, WKT):
                        w1 = min(w0 + WKT, nk)
                        sc = ps_score.tile([P, WKT * QCW], F32, tag="score",
                                           name="sc")
                        # trim: fp32r pays 4x below 256 moving cols, so each
                        # tile keeps >=256; FINETRIM trims per-tile (the exp
                        # of a mixed wave splits into per-tile instructions
                        # so no unwritten PSUM is ever read)
                        if FINETRIM:
                            oms = []
                            for kt in range(w0, w1):
                                o = max(0, kt * P - q0) if TRIM else 0
                                if QCW - o < 256:
                                    o = QCW - 256
                                oms.append(o)
                        else:
                            om = QCW - 256 if TRIM else 0
                            for kt in range(w0, w1):
                                om = min(om, max(0, kt * P - q0))
                            oms = [om] * (w1 - w0)
                        for kt in range(w0, w1):
                            j = kt - w0
                            nc.tensor.matmul(
                                sc[:, j * QCW + oms[j]:(j + 1) * QCW],
                                _r(kT_sb[prow, hp * S + kt * P: hp * S + (kt + 1) * P]),
                                _r(qT_sb[prow, hp * S + q0 + oms[j]: hp * S + q0 + QCW]),
                                start=True, stop=True)
                        nw = (w1 - w0) * QCW
                        pw = pw_pool.tile([P, WKT * QCW], F32R, tag="pw", name="pw")
                        om = oms[0]
                        if len(set(oms)) > 1:
                            sc3 = sc[:].rearrange("p (t c) -> p t c", c=QCW)
                            pw3 = pw[:].rearrange("p (t c) -> p t c", c=QCW)
                            for j, o in enumerate(oms):
                                nc.scalar.activation(
                                    pw3[:, j:j + 1, o:QCW],
                                    sc3[:, j:j + 1, o:QCW], Exp)
                        elif om:
                            sc3 = sc[:].rearrange("p (t c) -> p t c", c=QCW)
                            pw3 = pw[:].rearrange("p (t c) -> p t c", c=QCW)
                            nc.scalar.activation(
                                pw3[:, 0:w1 - w0, om:QCW],
                                sc3[:, 0:w1 - w0, om:QCW], Exp)
                        else:
                            nc.scalar.activation(pw[:, :nw], sc[:, :nw], Exp)
                        for kt in range(w0, w1):
                            o = kt * P - q0
                            if o >= 0:  # diagonal tile: mask mixed block
                                j = kt - w0
                                blk = slice(j * QCW + o, j * QCW + o + P)
                                getattr(nc, mask_eng).tensor_tensor(
                                    out=pw[:, blk], in0=pw[:, blk],
                                    in1=tri_sb[:], op=MULT)
                        pend.append((pw, w0, w1))
                        if len(pend) > LAG:
                            drain_prepv()
                            emit_pv(st, *pend.pop(0))
                        if prepv:
                            prepv.pop(0)()
                        cred[0] += pace
                        while cred[0] >= 1.0 and fillq:
                            fillq.pop(0)()
                            cred[0] -= 1.0
                    drain_prepv()
                    # optionally interleave leftover fill into the flush/norm
                    # chain so the PE isn't idle during recip/bcast latency
                    tf = TAILFILL and qc == NQC - 1
                    for item in pend:
                        emit_pv(st, *item)
                        if tf and fillq:
                            fillq.pop(0)()
                    if tf and fillq:
                        fillq.pop(0)()
                    emit_norm(st, pe_bcast=pe_bcast)
                    if tf and fillq:
                        fillq.pop(0)()

                def evac_store(ps, oc, s0, eng='vector', dst=None):
                    ost = outst_pool.tile([P, QCW], OUT_DT, tag="outst",
                                          name="ost")
                    if eng == 'alt':
                        eng = 'vector' if oc % 2 else 'scalar'
                    if eng == 'split':
                        nc.vector.tensor_copy(ost[:, 0:QCW // 2],
                                              ps[:, 0:QCW // 2])
                        nc.scalar.activation(ost[:, QCW // 2:QCW],
                                             ps[:, QCW // 2:QCW], Copy)
                        q = nc.sync if oc % 2 else dmaq
                    elif eng == 'vector':
                        nc.vector.tensor_copy(ost[:], ps[:])
                        q = dmaq
                    else:
                        nc.scalar.activation(ost[:], ps[:], Copy)
                        q = dmaq
                    if dst is None:
                        q.dma_start(
                            out=outT_d[oc * P:(oc + 1) * P, s0:s0 + QCW],
                            in_=ost[:])
                    else:
                        q.dma_start(out=dst[oc * P:(oc + 1) * P, 0:QCW],
                                    in_=ost[:])

                def out_groups(qc):
                    # output projection for query chunk qc (one 512-col slab):
                    # one thunk per 128-row output chunk; stores batched in
                    # oc-pairs (halves the store-DMA count / tail drain)
                    shared = {}
                    def g(oc, s0=qc * QCW):
                        ps = ps_o.tile([P, QCW], F32, tag="oT", name="ps_c")
                        for ac in range(2):
                            nc.tensor.matmul(
                                ps[:],
                                _r(wo_sb[:, ac * D + oc * P: ac * D + (oc + 1) * P]),
                                _r(oTn_sb[:, ac * S + s0: ac * S + s0 + QCW]),
                                start=(ac == 0), stop=(ac == 1))
                        pair = PAIR_STORE or (PAIR_LAST and qc == NQC - 1)
                        if not pair:
                            eng = (EVAC_LAST if EVAC_LAST and qc == NQC - 1
                                   else EVAC_ENG)
                            evac_store(ps, oc, s0, eng=eng)
                            return
                        if oc % 2 == 0:
                            shared["ost"] = outst_pool.tile(
                                [P, 2 * QCW], OUT_DT, tag="outst", name="ost2")
                        ost = shared["ost"]
                        half = ost[:, (oc % 2) * QCW:(oc % 2 + 1) * QCW]
                        eng = (EVAC_LAST if EVAC_LAST and qc == NQC - 1
                               else EVAC_ENG)
                        if eng == 'alt':
                            eng = 'vector' if oc % 2 else 'scalar'
                        if eng == 'scalar':
                            nc.scalar.activation(half, ps[:], Copy)
                        else:
                            nc.vector.tensor_copy(half, ps[:])
                        if oc % 2 == 1:
                            dmaq.dma_start(
                                out=outT_d.rearrange("(oc p) s -> p oc s", p=P)
                                [:, oc - 1:oc + 1, s0:s0 + QCW],
                                in_=ost[:].rearrange("p (two s) -> p two s",
                                                     two=2))
                    return [lambda o=oc: g(o) for oc in range(KC)]

                def out_half_groups(qc, ac):
                    # first contraction half of chunk qc's out-projection,
                    # evacuated to outhalf_sb (finished by out_final_groups)
                    def g(oc, s0=qc * QCW):
                        ps = ps_o.tile([P, QCW], F32, tag="oT", name="ps_h")
                        nc.tensor.matmul(
                            ps[:],
                            _r(wo_sb[:, ac * D + oc * P: ac * D + (oc + 1) * P]),
                            _r(oTn_sb[:, ac * S + s0: ac * S + s0 + QCW]),
                            start=True, stop=True)
                        evac_store(ps, oc, s0, eng=EVAC_ENG)
                    return [lambda o=oc: g(o) for oc in range(KC)]

                def out_final_groups(qc, ac):
                    def g(oc, s0=qc * QCW):
                        ps = ps_o.tile([P, QCW], F32, tag="oT", name="ps_f")
                        nc.tensor.matmul(
                            ps[:],
                            _r(wo_sb[:, ac * D + oc * P: ac * D + (oc + 1) * P]),
                            _r(oTn_sb[:, ac * S + s0: ac * S + s0 + QCW]),
                            start=True, stop=True)
                        evac_store(ps, oc, s0, eng='split', dst=outT2_d)
                    return [lambda o=oc: g(o) for oc in range(KC)]

                # ---- pipelined emission ----
                # unit (n, lh) runs with fill work popped between waves:
                # projection chunk n+1 plus out-projections per OUTQ.
                # Only the q/k halves the first unit reads run upfront; the
                # other halves go to fillq and the v-groups run as pre-PV
                # fill inside the first unit, so the first exp fires ~12us
                # earlier.
                def warm(k):
                    # junk matmuls on resident tri keep the PE p-state ramp
                    # anchored across known dependency gaps
                    for _ in range(k):
                        wps = ps_o.tile([P, QCW], F32, tag="oT", name="warm")
                        nc.tensor.matmul(wps[:, 0:P], tri_sb[:].bitcast(F32R),
                                         tri_sb[:].bitcast(F32R),
                                         start=True, stop=True)

                gs0 = proj_groups(0)
                first_m = HORD[0] // 2
                fillq = [gs0[1 - first_m], gs0[3 - first_m]]
                prepv0 = gs0[4:]
                warm(WARM_HEAD)
                for g in (gs0[first_m], gs0[2 + first_m]):
                    g()
                last_qc = NQC - 1
                hp_pos = {}
                for i, lh in enumerate(HORD):
                    hp_pos[lh // 2] = i
                first_hp = min(hp_pos, key=lambda h: hp_pos[h])
                for n in range(NQC):
                    if XSPLIT and _rep == 0:
                        # late x chunks / wo issued just-in-time so mid-flight
                        # DMAs never queue behind multi-MB input transfers
                        if n + 1 < NQC:
                            load_x(n + 1)
                        if n == 1:
                            load_wo(0)
                            load_wo(1)
                    if n + 1 < NQC:
                        fillq.extend(proj_groups(n + 1))
                    for oq in OUTQ[n]:
                        fillq.extend(out_groups(oq))
                    waves_u = -(-HPC * (n + 1) // WKT)
                    for i, lh in enumerate(HORD):
                        # reserve a few thunks on the final unit for the
                        # flush/norm interleave above
                        resv = RESV if (n == last_qc and i == HPC - 1) else 0
                        pace = len(fillq) / max((HPC - i) * waves_u + resv, 1)
                        cred = [0.0]
                        is_last = (n == last_qc and i == HPC - 1)
                        is_l2 = (n == last_qc and i >= HPC - 2)
                        emit_attn_unit(n, lh, fillq, pace, cred,
                                       pe_bcast=(BCPE == 'all' or
                                                 (BCPE == 'last' and is_last) or
                                                 (BCPE == 'last2' and is_l2)),
                                       prepv=(prepv0 if n == 0 and i == 0
                                              else None))
                        if n == last_qc and i >= HPC - 2:
                            warm(WARM_TAIL)
                        if dummy:
                            emit_dummies()
                        if (n == last_qc and SPLIT_LAST
                                and i == min(hp_pos[first_hp] + 1, HPC - 1)):
                            fillq.extend(out_half_groups(n, first_hp))
                for g in fillq:
                    g()
                if SPLIT_LAST:
                    for g in out_final_groups(last_qc, 1 - first_hp):
                        g()
                else:
                    for g in out_groups(last_qc):
                        g()
            if dbg:
                for m in range(2):
                    nc.sync.dma_start(out=dbg_q[m * P:(m + 1) * P, :],
                                      in_=qT_sb[:, m * S:(m + 1) * S].bitcast(F32))
                    nc.sync.dma_start(out=dbg_k[m * P:(m + 1) * P, :],
                                      in_=kT_sb[:, m * S:(m + 1) * S].bitcast(F32))
                    nc.sync.dma_start(out=dbg_o[m * P:(m + 1) * P, :],
                                      in_=oTn_sb[:, m * S:(m + 1) * S].bitcast(F32))
                nc.sync.dma_start(out=dbg_v[:], in_=v_sb[:].bitcast(F32))

        for _f in reversed(_frees):
            _f()

    nc.compile()
    return nc


def make_in_maps(x, Wq, bq, Wk, bk, Wv, bv, Wo, bf16_in=True):
    """Host-side sharding: per-core input dicts."""
    import ml_dtypes
    tri = (np.arange(P)[None, :] >= np.arange(P)[:, None]).astype(np.float32)
    f32c = lambda a: np.ascontiguousarray(a, dtype=np.float32)
    bf16c = lambda a: np.ascontiguousarray(a, dtype=ml_dtypes.bfloat16)
    in_maps = []
    for c in range(NCORES):
        b = c // CPB
        hb = c % CPB
        sl = slice(hb * DH, (hb + 1) * DH)
        m = {
            "woT": f32c(Wo[:, sl].T),
            "bq2": f32c(bq[sl].reshape(2, P).T),
            "bk2": f32c(bk[sl].reshape(2, P).T),
            "bv1": f32c(bv[sl].reshape(1, DH)),
            "tri": tri,
            "one64": np.ones((1, NKT * HPC), np.float32),
        }
        if bf16_in:
            m["xT16"] = bf16c(np.asarray(x[b]).T)
            m["wqT16"] = bf16c(np.asarray(Wq[sl, :]).T)
            m["wkT16"] = bf16c(np.asarray(Wk[sl, :]).T)
            m["wvT16"] = bf16c(np.asarray(Wv[sl, :]).T)
        else:
            m["xT"] = f32c(x[b].T)
            m["wqT"] = f32c(Wq[sl, :].T)
            m["wkT"] = f32c(Wk[sl, :].T)
            m["wvT"] = f32c(Wv[sl, :].T)
        in_maps.append(m)
    return in_maps


def kernel(x, mask, Wq, bq, Wk, bk, Wv, bv, Wo, bo, **unused):
    if "nc" not in _CACHE:
        _CACHE["nc"] = build(**BEST)
    nc = _CACHE["nc"]
    x = np.asarray(x)
    in_maps = make_in_maps(np.asarray(x), np.asarray(Wq), np.asarray(bq),
                           np.asarray(Wk), np.asarray(bk), np.asarray(Wv),
                           np.asarray(bv), np.asarray(Wo))
    res = run_bass_kernel_spmd(nc, in_maps, list(range(NCORES)))
    out = np.zeros((B, S, D), dtype=np.float32)
    for c in range(NCORES):
        out[c // CPB] += np.asarray(res.results[c]["outT"],
                                    dtype=np.float32).T
        if BEST.get("SPLIT_LAST", True):
            out[c // CPB][(NQC - 1) * QCW:] += np.asarray(
                res.results[c]["outT2"], dtype=np.float32).T
    out += np.asarray(bo, dtype=np.float32)[None, None, :]
    return out

